# revision 7
# baseline (speedup 1.0000x reference)
"""Trainium2 Bass kernel for nn_ConcentrationLoss — v2 (fp32r matmuls).

Math per (b, c) slice of pred/target [B,C,H,W]:
    mass = sum(t); cy = sum(t*y)/mass; cx = sum(t*x)/mass
    per_slice = mean(sigmoid(pred) * ((y-cy)^2 + (x-cx)^2))
    loss = mean(per_slice over slices with mass > 0)

Per slice the device computes streaming moment sums with centered coords
y' = y-(H-1)/2, x' = x-(W-1)/2:
    [T0, S0, Ty', Sy', Sy'y'] via TensorE matmuls (weight cols [1,y',y'^2])
    contracting the h/partition axis into a [5, 512] fp32 PSUM stripe; then
    VectorE x'-multiplies + free-axis reduces for Tx', Sx', Sx'x'.

v2 differences vs baseline:
  - Matmul rhs is raw fp32 bitcast to float32r (full PE rate at N=512):
    no bf16 cast of target (saves the whole DVE copy pass), sigmoid
    writes fp32 directly.
  - 3 slices packed per PSUM bank at partition offsets {0,32,64}; the
    x-moment stage runs once per bank on [128,512] (5 DVE ops per 3
    slices instead of ~5 per slice).
  - Const DMAs issued without serializing waits; 5-deep SBUF pipeline.
Data-parallel on batch across 8 cores (20 slices each); scalar tail
(centroid divides + masked mean) combined on the host in float64.
"""

import sys

for _p in ("/opt/trn_rl_repo",):
    if _p not in sys.path:
        sys.path.append(_p)

import numpy as np

import concourse.bass as bass
from concourse import mybir
from concourse.bass_utils import run_bass_kernel_spmd

B, C, H, W = 16, 10, 512, 512
NCORES = 8
BPC = B // NCORES          # batches per core
S = BPC * C                # slices per core (20)
NCHUNK = H // 128          # 4 h-chunks per slice
FW = NCHUNK * W            # free size of a full-slice SBUF tile (2048)
HW = float(H * W)
D = 5                      # SBUF pipeline depth (slices in flight)
SPB = 3                    # slices per PSUM bank (offsets 0/32/64)
NBANK = (S + SPB - 1) // SPB  # PSUM banks (7)

_CACHE = {}
ISSUE_MODE = "multi"   # "multi": DMAs split over SP/Act/Pool; "sp": all on SP
USE_IOTA = True        # build xc/xc2 on device (else DMA them)
USE_ACT_ACCUM = False  # broken on HW: Sigmoid->Copy act-table switch faults at runtime
USE_CHUNK19 = True     # last target slice as 4 chunk DMAs


WWIN = 32 * (SPB - 1) + 5      # matmul M: covers SPB 5-row bands at 32-offsets


def _widx(q, kind, band):
    """Column base of the weight window for (h-chunk q, kind 0=target
    1=pred, psum band)."""
    return ((q * 2 + kind) * SPB + band) * WWIN


def _moment_weights():
    """[128, NCHUNK*2*SPB*WWIN] fp32. Row h = 4*j + q maps to partition j,
    chunk q (so each partition's DMA run is 8KB contiguous). fp32r matmuls
    only allow PSUM base partition 0, so each (q, kind, band) gets an
    M=WWIN window whose 5 nonzero columns sit at partition 32*band: target
    -> rows [t, 0, t*y', 0, 0], pred -> rows [0, s, 0, s*y', s*y'^2], with
    y'(j, q) = 4j + q - (H-1)/2. Zero columns contribute zero to the other
    bands, so the SPB slices of a bank accumulate independently in one
    accumulation group."""
    w = np.zeros((128, NCHUNK * 2 * SPB * WWIN), dtype=np.float32)
    for q in range(NCHUNK):
        yp = ((4.0 * np.arange(128, dtype=np.float64) + q) - (H - 1) / 2.0).astype(
            np.float32
        )
        for b in range(SPB):
            t0 = _widx(q, 0, b) + 32 * b
            w[:, t0 + 0] = 1.0
            w[:, t0 + 2] = yp
            p0 = _widx(q, 1, b) + 32 * b
            w[:, p0 + 1] = 1.0
            w[:, p0 + 3] = yp
            w[:, p0 + 4] = yp * yp
    return w


def _build_nc():
    nc = bass.Bass("TRN2", target_bir_lowering=False, debug=False)
    f32, f32r = mybir.dt.float32, mybir.dt.float32r

    pred_d = nc.dram_tensor("pred", [S, H, W], f32, kind="ExternalInput")
    targ_d = nc.dram_tensor("target", [S, H, W], f32, kind="ExternalInput")
    wmom_d = nc.dram_tensor("wmom", [128, NCHUNK * 2 * SPB * WWIN], f32,
                            kind="ExternalInput")
    if not USE_IOTA:
        xc_d = nc.dram_tensor("xc", [128, W], f32, kind="ExternalInput")
        xc2_d = nc.dram_tensor("xc2", [128, W], f32, kind="ExternalInput")
    out_d = nc.dram_tensor("moments", [128, 3 * NBANK], f32, kind="ExternalOutput")

    pf = [nc.alloc_sbuf_tensor(f"pf{d}", [128, FW], f32) for d in range(D)]
    tf = [nc.alloc_sbuf_tensor(f"tf{d}", [128, FW], f32r) for d in range(D)]
    sf = [nc.alloc_sbuf_tensor(f"sf{d}", [128, FW], f32r) for d in range(D)]
    wsb = nc.alloc_sbuf_tensor("wsb", [128, NCHUNK * 2 * SPB * WWIN], f32r)
    xi = nc.alloc_sbuf_tensor("xi", [128, W], f32)
    xcsb = nc.alloc_sbuf_tensor("xcsb", [128, W], f32)
    xc2sb = nc.alloc_sbuf_tensor("xc2sb", [128, W], f32)
    t1 = nc.alloc_sbuf_tensor("t1", [128, W], f32)
    t2 = nc.alloc_sbuf_tensor("t2", [128, W], f32)
    t3 = nc.alloc_sbuf_tensor("t3", [128, W], f32)
    O = nc.alloc_sbuf_tensor("O", [128, 3 * NBANK], f32)
    ps = [nc.alloc_psum_tensor(f"ps{i}", [128, W], f32) for i in range(NBANK)]

    cW = nc.alloc_semaphore("cW")        # wmom DMA (16)
    cX = nc.alloc_semaphore("cX")        # xc ready (1)
    cX2 = nc.alloc_semaphore("cX2")      # xc2 ready (1)
    gsem = nc.alloc_semaphore("gsem")    # iota done (1)
    # Input DMAs are spread over three issuing engines (SP / Pool / DVE) so
    # their DGE queues pipeline independently. Per-substream parity
    # semaphore pairs; each DMA is issue-gated on its substream predecessor
    # two back (establishes completion order for consumers' waits).
    pp = [nc.alloc_semaphore(f"pp{b}") for b in range(2)]  # SP preds
    pd = [nc.alloc_semaphore(f"pd{b}") for b in range(2)]  # DVE preds
    st = [nc.alloc_semaphore(f"st{b}") for b in range(2)]  # SP targets
    pt = [nc.alloc_semaphore(f"pt{b}") for b in range(2)]  # Pool targets
    tch = [nc.alloc_semaphore(f"tch{k}") for k in range(NCHUNK)]  # last-slice target chunks
    asem = nc.alloc_semaphore("asem")    # sigmoid done (1 per slice)
    peS = nc.alloc_semaphore("peS")      # matmul group done (1 per slice)
    dst2 = nc.alloc_semaphore("dst2")    # stage2 done (1 per bank)
    osem = nc.alloc_semaphore("osem")    # out DMA
    msem = nc.alloc_semaphore("msem")    # psum bank memset done (1 per bank)
    vsem = nc.alloc_semaphore("vsem")    # DVE same-engine drain chain
    a2 = nc.alloc_semaphore("a2")        # Act bank reduce done (1 per bank)

    def dram_slice(t, s):
        # h = 4*p + q: partition p takes 4 consecutive rows = one 8KB
        # contiguous DMA descriptor per partition (full 360GB/s rate).
        # Keep the AP 2D so the innermost run is the full 8KB.
        return t[s].rearrange("(p q) w -> p (q w)", q=NCHUNK)

    # Stream assignment across the three DMA-capable engines (SP/Act/Pool):
    # Act self-feeds 4 mid-run preds (it has slack besides sigmoids), SP
    # carries the rest of the preds + one early target, Pool carries the
    # target stream (slice S-1 is chunk-DMA'd). Pool's stream ends ~3us
    # after SP's so the last sigmoid + pred matmuls hide under it.
    if ISSUE_MODE == "multi":
        act_pred = [3, 7, 11, 15, 19]  # self-fed ~4 slices ahead of their sigmoid
        sp_targ = [16, 18]         # issued after SP's preds; fill SP's stream end
    else:
        act_pred = []
        sp_targ = []
    pred_eng = {s: ("act" if s in act_pred else "sp") for s in range(S)}
    targ_eng = {s: ("sp" if s in sp_targ else "pool") for s in range(S)}
    sp_pred = [s for s in range(S) if pred_eng[s] == "sp"]
    pool_targ = [s for s in range(S) if targ_eng[s] == "pool"]
    pred_sub = {}   # slice -> (sem pair, index in substream)
    for lst, sems in ((sp_pred, pp), (act_pred, pd)):
        for k, s in enumerate(lst):
            pred_sub[s] = (sems, k)
    targ_sub = {}
    if ISSUE_MODE == "multi":
        for lst, sems in ((sp_targ, st), (pool_targ, pt)):
            for k, s in enumerate(lst):
                targ_sub[s] = (sems, k)
    else:
        for k, s in enumerate(range(S)):
            targ_sub[s] = (pt, k)

    def issue_pred(eng, s):
        sems, k = pred_sub[s]
        if k >= 2:
            eng.wait_ge(sems[k % 2], 16 * (k // 2))  # substream issue order
        if s >= D:
            eng.wait_ge(asem, s - D + 1)             # pf slot drained by sigmoid
        eng.dma_start(pf[s % D][:], dram_slice(pred_d, s)).then_inc(sems[k % 2], 16)

    def issue_targ(eng, s):
        if s >= D:
            eng.wait_ge(peS, s - D + 1)              # tf slot drained by matmuls
        if s == S - 1 and USE_CHUNK19:
            # last slice: 4 chunk DMAs (single-shot sems, no gating) so the
            # final target matmuls pipeline with the arriving chunks
            src3 = targ_d[s].rearrange("(p q) w -> p q w", q=NCHUNK)
            for k in range(NCHUNK):
                eng.dma_start(
                    tf[s % D][:, k * W : (k + 1) * W], src3[:, k, :].bitcast(f32r)
                ).then_inc(tch[k], 16)
            return
        sems, k = targ_sub[s]
        if k >= 2:
            eng.wait_ge(sems[k % 2], 16 * (k // 2))
        eng.dma_start(
            tf[s % D][:], dram_slice(targ_d, s).bitcast(f32r)
        ).then_inc(sems[k % 2], 16)

    with nc.Block() as block:

        @block.sync
        def _(sync):
            if ISSUE_MODE == "multi":
                for s in sp_pred:
                    issue_pred(sync, s)
                for s in sp_targ:
                    issue_targ(sync, s)
            else:
                for s in range(S):
                    issue_pred(sync, s)
                    if s == 0:
                        sync.dma_start(wsb[:], wmom_d[:].bitcast(f32r)).then_inc(cW, 16)
                        if not USE_IOTA:
                            sync.dma_start(xcsb[:], xc_d[:]).then_inc(cX, 16)
                            sync.dma_start(xc2sb[:], xc2_d[:]).then_inc(cX2, 16)
                    issue_targ(sync, s)
            sync.wait_ge(dst2, NBANK)
            if USE_ACT_ACCUM:
                sync.wait_ge(a2, 2)
            sync.dma_start(out_d[:], O[:]).then_inc(osem, 16)
            sync.wait_ge(osem, 16)

        @block.scalar
        def _(scalar):
            def bank_reduce(i):
                # free-axis sum of the last psum banks via Copy+accumulate;
                # runs parallel to the DVE x-moment ttrs in the kernel tail
                # (Copy shares Sigmoid's act table - no reload)
                scalar.wait_ge(peS, min(SPB * (i + 1), S))
                if i > NBANK - 2:
                    scalar.wait_ge(a2, i - (NBANK - 2))  # t3 drained (in-order)
                scalar.activation(
                    t3[:], ps[i][:], mybir.ActivationFunctionType.Copy,
                    accum_out=O[:, 3 * i : 3 * i + 1],
                ).then_inc(a2, 1)

            if ISSUE_MODE == "multi":
                scalar.dma_start(wsb[:], wmom_d[:].bitcast(f32r)).then_inc(cW, 16)
                if not USE_IOTA:
                    scalar.dma_start(xcsb[:], xc_d[:]).then_inc(cX, 16)
                    scalar.dma_start(xc2sb[:], xc2_d[:]).then_inc(cX2, 16)
            for sa in act_pred:
                if sa < D:
                    issue_pred(scalar, sa)  # idle window before pred 0 lands
            for s in range(S):
                # issue own pred DMAs ~5 slices ahead (slot s' % D just freed)
                for sa in act_pred:
                    if sa >= D and sa - D == s - 1:
                        issue_pred(scalar, sa)
                sems, k = pred_sub[s]
                scalar.wait_ge(sems[k % 2], 16 * (k // 2 + 1))
                if s >= D:
                    scalar.wait_ge(peS, s - D + 1)   # sf slot drained by matmuls
                scalar.activation(
                    sf[s % D][:], pf[s % D][:], mybir.ActivationFunctionType.Sigmoid
                ).then_inc(asem, 1)
            if USE_ACT_ACCUM:
                bank_reduce(NBANK - 2)
                bank_reduce(NBANK - 1)

        @block.gpsimd
        def _(gpsimd):
            # xc = (0..W-1) - (W-1)/2 and xc2 = xc^2, built on-device so the
            # DMA streams carry no big constant transfers.
            if USE_IOTA:
                gpsimd.iota(
                    xi[:], pattern=[[1, W]], base=0, channel_multiplier=0,
                    allow_small_or_imprecise_dtypes=True,
                ).then_inc(gsem, 1)
            if ISSUE_MODE == "multi":
                for s in pool_targ:
                    issue_targ(gpsimd, s)

        @block.vector
        def _(vector):
            for i in range(NBANK):
                vector.memset(ps[i][:], 0.0).then_inc(msem, 1)
            if USE_IOTA:
                vector.wait_ge(gsem, 1)
                vector.tensor_scalar_add(xcsb[:], xi[:], -(W - 1) / 2.0).then_inc(cX, 1)
                vector.wait_ge(cX, 1)
                vector.tensor_mul(xc2sb[:], xcsb[:], xcsb[:]).then_inc(cX2, 1)
                vector.wait_ge(cX2, 1)
            else:
                vector.wait_ge(cX, 16)
                vector.wait_ge(cX2, 16)
            vcnt = [0]

            def vstep(inst):
                inst.then_inc(vsem, 1)
                vcnt[0] += 1

            for i in range(NBANK):
                p = ps[i]
                vector.wait_ge(peS, min(SPB * (i + 1), S))
                if (i < NBANK - 2) or not USE_ACT_ACCUM:
                    # plain column sums; the last two banks' go to Act so the
                    # tail runs DVE and Act in parallel
                    vector.reduce_sum(
                        O[:, 3 * i : 3 * i + 1], p[:, :], axis=mybir.AxisListType.X
                    )
                if i >= 1:
                    vector.wait_ge(vsem, vcnt[0])    # t1/t2 readers drained
                vstep(vector.tensor_mul(t1[:], p[:, :], xcsb[:]))
                vector.wait_ge(vsem, vcnt[0])
                vstep(vector.reduce_sum(
                    O[:, 3 * i + 1 : 3 * i + 2], t1[:], axis=mybir.AxisListType.X
                ))
                vstep(vector.tensor_mul(t2[:], p[:, :], xc2sb[:]))
                vector.wait_ge(vsem, vcnt[0])
                vector.reduce_sum(
                    O[:, 3 * i + 2 : 3 * i + 3], t2[:], axis=mybir.AxisListType.X
                ).then_inc(dst2, 1)

        @block.tensor
        def _(tensor):
            tensor.wait_ge(cW, 16)                   # wsb loaded
            tensor.wait_ge(msem, NBANK)              # psum banks zeroed
            for s in range(S):
                d = s % D
                p = ps[s // SPB]
                b = s % SPB
                last_of_bank = (s == S - 1) or (b == SPB - 1)
                tensor.wait_ge(asem, s + 1)
                for k in range(NCHUNK):
                    w0 = _widx(k, 1, b)
                    tensor.matmul(
                        p[0:WWIN, :],
                        wsb[:, w0 : w0 + WWIN],
                        sf[d][:, k * W : (k + 1) * W],
                        start=(b == 0 and k == 0),
                        stop=False,
                    )
                for k in range(NCHUNK):
                    if s == S - 1 and USE_CHUNK19:
                        tensor.wait_ge(tch[k], 16)
                    elif k == 0:
                        sems, kk = targ_sub[s]
                        tensor.wait_ge(sems[kk % 2], 16 * (kk // 2 + 1))
                    w0 = _widx(k, 0, b)
                    mm = tensor.matmul(
                        p[0:WWIN, :],
                        wsb[:, w0 : w0 + WWIN],
                        tf[d][:, k * W : (k + 1) * W],
                        start=False,
                        stop=(last_of_bank and k == NCHUNK - 1),
                    )
                mm.then_inc(peS, 1)

    return nc


def _host_consts():
    if USE_IOTA:
        return _moment_weights()
    xp = (np.arange(W, dtype=np.float64) - (W - 1) / 2.0).astype(np.float32)
    xc = np.broadcast_to(xp, (128, W)).copy()
    xc2 = np.broadcast_to((xp.astype(np.float64) ** 2).astype(np.float32),
                          (128, W)).copy()
    return (_moment_weights(), xc, xc2)


def _get_built():
    if "nc" not in _CACHE:
        _CACHE["nc"] = _build_nc()
        _CACHE["consts"] = _host_consts()
    return _CACHE["nc"], _CACHE["consts"]


def _combine(moments_per_core):
    loss_sum = 0.0
    n_valid = 0
    for O in moments_per_core:
        O = np.asarray(O, dtype=np.float64)
        for s in range(S):
            i, q = s // SPB, s % SPB
            p0 = 32 * q
            T0 = O[p0 + 0, 3 * i]
            S0 = O[p0 + 1, 3 * i]
            Ty = O[p0 + 2, 3 * i]
            Sy = O[p0 + 3, 3 * i]
            Syy = O[p0 + 4, 3 * i]
            Tx = O[p0 + 0, 3 * i + 1]
            Sx = O[p0 + 1, 3 * i + 1]
            Sxx = O[p0 + 1, 3 * i + 2]
            if T0 > 0:
                cy = Ty / T0
                cx = Tx / T0
                loss_sum += (
                    (Syy - 2.0 * cy * Sy + cy * cy * S0)
                    + (Sxx - 2.0 * cx * Sx + cx * cx * S0)
                ) / HW
                n_valid += 1
    if n_valid > 0:
        return np.float32(loss_sum / n_valid)
    return np.float32(0.0)


def _in_maps(pred, target, consts):
    if USE_IOTA:
        base = {"wmom": consts}
    else:
        base = {"wmom": consts[0], "xc": consts[1], "xc2": consts[2]}
    maps = []
    for i in range(NCORES):
        maps.append(
            {
                "pred": pred[i * BPC : (i + 1) * BPC].reshape(S, H, W),
                "target": target[i * BPC : (i + 1) * BPC].reshape(S, H, W),
                **base,
            }
        )
    return maps


def kernel(pred, target):
    pred = np.ascontiguousarray(np.asarray(pred, dtype=np.float32))
    target = np.ascontiguousarray(np.asarray(target, dtype=np.float32))
    assert pred.shape == (B, C, H, W) and target.shape == (B, C, H, W)

    nc, consts = _get_built()
    res = run_bass_kernel_spmd(nc, _in_maps(pred, target, consts),
                               list(range(NCORES)))
    outs = [res.results[i]["moments"] for i in range(NCORES)]
    return np.asarray(_combine(outs), dtype=np.float32)


if __name__ == "__main__":
    rng = np.random.default_rng(0)
    p = rng.standard_normal((B, C, H, W), dtype=np.float32)
    t = rng.random((B, C, H, W), dtype=np.float32)
    print(kernel(pred=p, target=t))


# revision 10
# speedup vs baseline: 1.0351x; 1.0351x over previous
"""Trainium2 Bass kernel for nn_ConcentrationLoss — v2 (fp32r matmuls).

Math per (b, c) slice of pred/target [B,C,H,W]:
    mass = sum(t); cy = sum(t*y)/mass; cx = sum(t*x)/mass
    per_slice = mean(sigmoid(pred) * ((y-cy)^2 + (x-cx)^2))
    loss = mean(per_slice over slices with mass > 0)

Per slice the device computes streaming moment sums with centered coords
y' = y-(H-1)/2, x' = x-(W-1)/2:
    [T0, S0, Ty', Sy', Sy'y'] via TensorE matmuls (weight cols [1,y',y'^2])
    contracting the h/partition axis into a [5, 512] fp32 PSUM stripe; then
    VectorE x'-multiplies + free-axis reduces for Tx', Sx', Sx'x'.

v2 differences vs baseline:
  - Matmul rhs is raw fp32 bitcast to float32r (full PE rate at N=512):
    no bf16 cast of target (saves the whole DVE copy pass), sigmoid
    writes fp32 directly.
  - 3 slices packed per PSUM bank at partition offsets {0,32,64}; the
    x-moment stage runs once per bank on [128,512] (5 DVE ops per 3
    slices instead of ~5 per slice).
  - Const DMAs issued without serializing waits; 5-deep SBUF pipeline.
Data-parallel on batch across 8 cores (20 slices each); scalar tail
(centroid divides + masked mean) combined on the host in float64.
"""

import sys

for _p in ("/opt/trn_rl_repo",):
    if _p not in sys.path:
        sys.path.append(_p)

import numpy as np

import concourse.bass as bass
from concourse import mybir
from concourse.bass_utils import run_bass_kernel_spmd

B, C, H, W = 16, 10, 512, 512
NCORES = 8
BPC = B // NCORES          # batches per core
S = BPC * C                # slices per core (20)
NCHUNK = H // 128          # 4 h-chunks per slice
FW = NCHUNK * W            # free size of a full-slice SBUF tile (2048)
HW = float(H * W)
D = 5                      # SBUF pipeline depth (slices in flight)
SPB = 3                    # slices per PSUM bank (offsets 0/32/64)
NBANK = (S + SPB - 1) // SPB  # PSUM banks (7)

_CACHE = {}
ISSUE_MODE = "multi"   # "multi": DMAs split over SP/Act/Pool; "sp": all on SP
USE_IOTA = True        # build xc/xc2 on device (else DMA them)
USE_ACT_ACCUM = False  # broken on HW: Sigmoid->Copy act-table switch faults at runtime
USE_CHUNK19 = True     # last target slice as 4 chunk DMAs
USE_POOL_RED6 = False  # gpsimd tensor_reduce is partition-axis only
USE_AMR = False        # affine_mul_reduce fails walrus InstISA codegen here
USE_RAW6 = True        # last psum bank shipped raw; host does its reductions


WWIN = 32 * (SPB - 1) + 5      # matmul M: covers SPB 5-row bands at 32-offsets


def _widx(q, kind, band):
    """Column base of the weight window for (h-chunk q, kind 0=target
    1=pred, psum band)."""
    return ((q * 2 + kind) * SPB + band) * WWIN


def _moment_weights():
    """[128, NCHUNK*2*SPB*WWIN] fp32. Row h = 4*j + q maps to partition j,
    chunk q (so each partition's DMA run is 8KB contiguous). fp32r matmuls
    only allow PSUM base partition 0, so each (q, kind, band) gets an
    M=WWIN window whose 5 nonzero columns sit at partition 32*band: target
    -> rows [t, 0, t*y', 0, 0], pred -> rows [0, s, 0, s*y', s*y'^2], with
    y'(j, q) = 4j + q - (H-1)/2. Zero columns contribute zero to the other
    bands, so the SPB slices of a bank accumulate independently in one
    accumulation group."""
    w = np.zeros((128, NCHUNK * 2 * SPB * WWIN), dtype=np.float32)
    for q in range(NCHUNK):
        yp = ((4.0 * np.arange(128, dtype=np.float64) + q) - (H - 1) / 2.0).astype(
            np.float32
        )
        for b in range(SPB):
            t0 = _widx(q, 0, b) + 32 * b
            w[:, t0 + 0] = 1.0
            w[:, t0 + 2] = yp
            p0 = _widx(q, 1, b) + 32 * b
            w[:, p0 + 1] = 1.0
            w[:, p0 + 3] = yp
            w[:, p0 + 4] = yp * yp
    return w


def _build_nc():
    nc = bass.Bass("TRN2", target_bir_lowering=False, debug=False)
    f32, f32r = mybir.dt.float32, mybir.dt.float32r

    pred_d = nc.dram_tensor("pred", [S, H, W], f32, kind="ExternalInput")
    targ_d = nc.dram_tensor("target", [S, H, W], f32, kind="ExternalInput")
    wmom_d = nc.dram_tensor("wmom", [128, NCHUNK * 2 * SPB * WWIN], f32,
                            kind="ExternalInput")
    if not USE_IOTA:
        xc_d = nc.dram_tensor("xc", [128, W], f32, kind="ExternalInput")
        xc2_d = nc.dram_tensor("xc2", [128, W], f32, kind="ExternalInput")
    out_d = nc.dram_tensor("moments", [128, 3 * NBANK], f32, kind="ExternalOutput")
    if USE_RAW6:
        raw6_d = nc.dram_tensor("raw6", [128, W], f32, kind="ExternalOutput")

    pf = [nc.alloc_sbuf_tensor(f"pf{d}", [128, FW], f32) for d in range(D)]
    tf = [nc.alloc_sbuf_tensor(f"tf{d}", [128, FW], f32r) for d in range(D)]
    sf = [nc.alloc_sbuf_tensor(f"sf{d}", [128, FW], f32r) for d in range(D)]
    wsb = nc.alloc_sbuf_tensor("wsb", [128, NCHUNK * 2 * SPB * WWIN], f32r)
    xi = nc.alloc_sbuf_tensor("xi", [128, W], f32)
    xcsb = nc.alloc_sbuf_tensor("xcsb", [128, W], f32)
    xc2sb = nc.alloc_sbuf_tensor("xc2sb", [128, W], f32)
    t1 = nc.alloc_sbuf_tensor("t1", [128, W], f32)
    t2 = nc.alloc_sbuf_tensor("t2", [128, W], f32)
    t3 = nc.alloc_sbuf_tensor("t3", [128, W], f32)
    O = nc.alloc_sbuf_tensor("O", [128, 3 * NBANK], f32)
    ps = [nc.alloc_psum_tensor(f"ps{i}", [128, W], f32) for i in range(NBANK)]

    cW = nc.alloc_semaphore("cW")        # wmom DMA (16)
    cX = nc.alloc_semaphore("cX")        # xc ready (1)
    cX2 = nc.alloc_semaphore("cX2")      # xc2 ready (1)
    gsem = nc.alloc_semaphore("gsem")    # iota done (1)
    # Input DMAs are spread over three issuing engines (SP / Pool / DVE) so
    # their DGE queues pipeline independently. Per-substream parity
    # semaphore pairs; each DMA is issue-gated on its substream predecessor
    # two back (establishes completion order for consumers' waits).
    pp = [nc.alloc_semaphore(f"pp{b}") for b in range(2)]  # SP preds
    pd = [nc.alloc_semaphore(f"pd{b}") for b in range(2)]  # DVE preds
    st = [nc.alloc_semaphore(f"st{b}") for b in range(2)]  # SP targets
    pt = [nc.alloc_semaphore(f"pt{b}") for b in range(2)]  # Pool targets
    tch = [nc.alloc_semaphore(f"tch{k}") for k in range(NCHUNK)]  # last-slice target chunks
    asem = nc.alloc_semaphore("asem")    # sigmoid done (1 per slice)
    peS = nc.alloc_semaphore("peS")      # matmul group done (1 per slice)
    dst2 = nc.alloc_semaphore("dst2")    # stage2 done (1 per bank)
    osem = nc.alloc_semaphore("osem")    # out DMA
    msem = nc.alloc_semaphore("msem")    # psum bank memset done (1 per bank)
    vsem = nc.alloc_semaphore("vsem")    # DVE same-engine drain chain
    pr6 = nc.alloc_semaphore("pr6")      # Pool bank-6 reduce done
    omz = nc.alloc_semaphore("omz")      # O tile zeroed
    a2 = nc.alloc_semaphore("a2")        # Act bank reduce done (1 per bank)

    def dram_slice(t, s):
        # h = 4*p + q: partition p takes 4 consecutive rows = one 8KB
        # contiguous DMA descriptor per partition (full 360GB/s rate).
        # Keep the AP 2D so the innermost run is the full 8KB.
        return t[s].rearrange("(p q) w -> p (q w)", q=NCHUNK)

    # Stream assignment across the three DMA-capable engines (SP/Act/Pool):
    # Act self-feeds 4 mid-run preds (it has slack besides sigmoids), SP
    # carries the rest of the preds + one early target, Pool carries the
    # target stream (slice S-1 is chunk-DMA'd). Pool's stream ends ~3us
    # after SP's so the last sigmoid + pred matmuls hide under it.
    if ISSUE_MODE == "multi":
        act_pred = [3, 7, 11, 15, 19]  # self-fed ~4 slices ahead of their sigmoid
        sp_targ = [16, 18]         # issued after SP's preds; fill SP's stream end
    else:
        act_pred = []
        sp_targ = []
    pred_eng = {s: ("act" if s in act_pred else "sp") for s in range(S)}
    targ_eng = {s: ("sp" if s in sp_targ else "pool") for s in range(S)}
    sp_pred = [s for s in range(S) if pred_eng[s] == "sp"]
    pool_targ = [s for s in range(S) if targ_eng[s] == "pool"]
    pred_sub = {}   # slice -> (sem pair, index in substream)
    for lst, sems in ((sp_pred, pp), (act_pred, pd)):
        for k, s in enumerate(lst):
            pred_sub[s] = (sems, k)
    targ_sub = {}
    if ISSUE_MODE == "multi":
        for lst, sems in ((sp_targ, st), (pool_targ, pt)):
            for k, s in enumerate(lst):
                targ_sub[s] = (sems, k)
    else:
        for k, s in enumerate(range(S)):
            targ_sub[s] = (pt, k)

    def issue_pred(eng, s):
        sems, k = pred_sub[s]
        if k >= 2:
            eng.wait_ge(sems[k % 2], 16 * (k // 2))  # substream issue order
        if s >= D:
            eng.wait_ge(asem, s - D + 1)             # pf slot drained by sigmoid
        eng.dma_start(pf[s % D][:], dram_slice(pred_d, s)).then_inc(sems[k % 2], 16)

    def issue_targ_chunks(eng, s, ks):
        # slice S-1 as per-chunk DMAs (single-shot sems, no gating) so the
        # final target matmuls pipeline with the arriving chunks
        src3 = targ_d[s].rearrange("(p q) w -> p q w", q=NCHUNK)
        for k in ks:
            eng.dma_start(
                tf[s % D][:, k * W : (k + 1) * W], src3[:, k, :].bitcast(f32r)
            ).then_inc(tch[k], 16)

    def issue_targ(eng, s):
        if s >= D:
            eng.wait_ge(peS, s - D + 1)              # tf slot drained by matmuls
        if s == S - 1 and USE_CHUNK19:
            issue_targ_chunks(eng, s, range(NCHUNK))
            return
        sems, k = targ_sub[s]
        if k >= 2:
            eng.wait_ge(sems[k % 2], 16 * (k // 2))
        eng.dma_start(
            tf[s % D][:], dram_slice(targ_d, s).bitcast(f32r)
        ).then_inc(sems[k % 2], 16)

    with nc.Block() as block:

        @block.sync
        def _(sync):
            if ISSUE_MODE == "multi":
                for s in sp_pred:
                    issue_pred(sync, s)
                for s in sp_targ:
                    issue_targ(sync, s)
            else:
                for s in range(S):
                    issue_pred(sync, s)
                    if s == 0:
                        sync.dma_start(wsb[:], wmom_d[:].bitcast(f32r)).then_inc(cW, 16)
                        if not USE_IOTA:
                            sync.dma_start(xcsb[:], xc_d[:]).then_inc(cX, 16)
                            sync.dma_start(xc2sb[:], xc2_d[:]).then_inc(cX2, 16)
                    issue_targ(sync, s)
            if USE_RAW6:
                sync.wait_ge(dst2, NBANK - 1)
                sync.wait_ge(omz, 1)
                sync.dma_start(out_d[:], O[:]).then_inc(osem, 16)
                sync.wait_ge(dst2, NBANK)     # t1 holds the raw bank copy
                sync.dma_start(raw6_d[:], t1[:]).then_inc(osem, 16)
                sync.wait_ge(osem, 32)
            else:
                sync.wait_ge(dst2, NBANK)
                if USE_POOL_RED6:
                    sync.wait_ge(pr6, 1)
                if USE_ACT_ACCUM:
                    sync.wait_ge(a2, 2)
                sync.dma_start(out_d[:], O[:]).then_inc(osem, 16)
                sync.wait_ge(osem, 16)

        @block.scalar
        def _(scalar):
            def bank_reduce(i):
                # free-axis sum of the last psum banks via Copy+accumulate;
                # runs parallel to the DVE x-moment ttrs in the kernel tail
                # (Copy shares Sigmoid's act table - no reload)
                scalar.wait_ge(peS, min(SPB * (i + 1), S))
                if i > NBANK - 2:
                    scalar.wait_ge(a2, i - (NBANK - 2))  # t3 drained (in-order)
                scalar.activation(
                    t3[:], ps[i][:], mybir.ActivationFunctionType.Copy,
                    accum_out=O[:, 3 * i : 3 * i + 1],
                ).then_inc(a2, 1)

            if ISSUE_MODE == "multi":
                scalar.dma_start(wsb[:], wmom_d[:].bitcast(f32r)).then_inc(cW, 16)
                if not USE_IOTA:
                    scalar.dma_start(xcsb[:], xc_d[:]).then_inc(cX, 16)
                    scalar.dma_start(xc2sb[:], xc2_d[:]).then_inc(cX2, 16)
            for sa in act_pred:
                if sa < D:
                    issue_pred(scalar, sa)  # idle window before pred 0 lands
            for s in range(S):
                # issue own pred DMAs ~5 slices ahead (slot s' % D just freed)
                for sa in act_pred:
                    if sa >= D and sa - D == s - 1:
                        issue_pred(scalar, sa)
                sems, k = pred_sub[s]
                scalar.wait_ge(sems[k % 2], 16 * (k // 2 + 1))
                if s >= D:
                    scalar.wait_ge(peS, s - D + 1)   # sf slot drained by matmuls
                scalar.activation(
                    sf[s % D][:], pf[s % D][:], mybir.ActivationFunctionType.Sigmoid
                ).then_inc(asem, 1)
            if USE_ACT_ACCUM:
                bank_reduce(NBANK - 2)
                bank_reduce(NBANK - 1)

        @block.gpsimd
        def _(gpsimd):
            # xc = (0..W-1) - (W-1)/2 and xc2 = xc^2, built on-device so the
            # DMA streams carry no big constant transfers.
            if USE_IOTA:
                gpsimd.iota(
                    xi[:], pattern=[[1, W]], base=0, channel_multiplier=0,
                    allow_small_or_imprecise_dtypes=True,
                ).then_inc(gsem, 1)
            if ISSUE_MODE == "multi":
                for s in pool_targ:
                    issue_targ(gpsimd, s)
            if USE_POOL_RED6:
                gpsimd.wait_ge(peS, S)
                gpsimd.reduce_sum(
                    O[:, 3 * (NBANK - 1) : 3 * (NBANK - 1) + 1],
                    ps[NBANK - 1][:, :], axis=mybir.AxisListType.X,
                ).then_inc(pr6, 1)

        @block.vector
        def _(vector):
            if USE_RAW6:
                vector.memset(O[:], 0.0).then_inc(omz, 1)
            for i in range(NBANK):
                vector.memset(ps[i][:], 0.0).then_inc(msem, 1)
            if USE_IOTA:
                vector.wait_ge(gsem, 1)
                vector.tensor_scalar_add(xcsb[:], xi[:], -(W - 1) / 2.0).then_inc(cX, 1)
                vector.wait_ge(cX, 1)
                vector.tensor_mul(xc2sb[:], xcsb[:], xcsb[:]).then_inc(cX2, 1)
                vector.wait_ge(cX2, 1)
            else:
                vector.wait_ge(cX, 16)
                vector.wait_ge(cX2, 16)
            vcnt = [0]

            def vstep(inst):
                inst.then_inc(vsem, 1)
                vcnt[0] += 1

            n_stage2 = NBANK - 1 if USE_RAW6 else NBANK
            for i in range(n_stage2):
                p = ps[i]
                vector.wait_ge(peS, min(SPB * (i + 1), S))
                pool_does = USE_POOL_RED6 and i == NBANK - 1
                act_does = USE_ACT_ACCUM and i >= NBANK - 2
                if not pool_does and not act_does:
                    # plain column sums; the last two banks' go to Act so the
                    # tail runs DVE and Act in parallel
                    vector.reduce_sum(
                        O[:, 3 * i : 3 * i + 1], p[:, :], axis=mybir.AxisListType.X
                    )
                if USE_AMR:
                    # fused (p*1+0)*xc -> t1, accum_out = sum: one DVE op per
                    # x-moment (production ant-dve op)
                    if i >= 1:
                        vector.wait_ge(vsem, vcnt[0])  # t1 writer drained
                        vector.wait_ge(dst2, i)        # t2 writer drained
                    vstep(vector.affine_mul_reduce(
                        t1[:], O[:, 3 * i + 1 : 3 * i + 2], p[:, :], xcsb[:],
                        1.0, 0.0,
                    ))
                    vector.affine_mul_reduce(
                        t2[:], O[:, 3 * i + 2 : 3 * i + 3], p[:, :], xc2sb[:],
                        1.0, 0.0,
                    ).then_inc(dst2, 1)
                else:
                    if i >= 1:
                        vector.wait_ge(vsem, vcnt[0])    # t1/t2 readers drained
                    vstep(vector.tensor_mul(t1[:], p[:, :], xcsb[:]))
                    vector.wait_ge(vsem, vcnt[0])
                    vstep(vector.reduce_sum(
                        O[:, 3 * i + 1 : 3 * i + 2], t1[:], axis=mybir.AxisListType.X
                    ))
                    vstep(vector.tensor_mul(t2[:], p[:, :], xc2sb[:]))
                    vector.wait_ge(vsem, vcnt[0])
                    vector.reduce_sum(
                        O[:, 3 * i + 2 : 3 * i + 3], t2[:], axis=mybir.AxisListType.X
                    ).then_inc(dst2, 1)
            if USE_RAW6:
                vector.wait_ge(peS, S)
                vector.wait_ge(vsem, vcnt[0])        # t1 free (bank reads done)
                vector.tensor_copy(t1[:], ps[NBANK - 1][:]).then_inc(dst2, 1)

        @block.tensor
        def _(tensor):
            tensor.wait_ge(cW, 16)                   # wsb loaded
            tensor.wait_ge(msem, NBANK)              # psum banks zeroed
            for s in range(S):
                d = s % D
                p = ps[s // SPB]
                b = s % SPB
                last_of_bank = (s == S - 1) or (b == SPB - 1)
                tensor.wait_ge(asem, s + 1)
                for k in range(NCHUNK):
                    w0 = _widx(k, 1, b)
                    tensor.matmul(
                        p[0:WWIN, :],
                        wsb[:, w0 : w0 + WWIN],
                        sf[d][:, k * W : (k + 1) * W],
                        start=(b == 0 and k == 0),
                        stop=False,
                    )
                for k in range(NCHUNK):
                    if s == S - 1 and USE_CHUNK19:
                        tensor.wait_ge(tch[k], 16)
                    elif k == 0:
                        sems, kk = targ_sub[s]
                        tensor.wait_ge(sems[kk % 2], 16 * (kk // 2 + 1))
                    w0 = _widx(k, 0, b)
                    mm = tensor.matmul(
                        p[0:WWIN, :],
                        wsb[:, w0 : w0 + WWIN],
                        tf[d][:, k * W : (k + 1) * W],
                        start=False,
                        stop=(last_of_bank and k == NCHUNK - 1),
                    )
                mm.then_inc(peS, 1)

    return nc


def _host_consts():
    if USE_IOTA:
        return _moment_weights()
    xp = (np.arange(W, dtype=np.float64) - (W - 1) / 2.0).astype(np.float32)
    xc = np.broadcast_to(xp, (128, W)).copy()
    xc2 = np.broadcast_to((xp.astype(np.float64) ** 2).astype(np.float32),
                          (128, W)).copy()
    return (_moment_weights(), xc, xc2)


def _get_built():
    if "nc" not in _CACHE:
        _CACHE["nc"] = _build_nc()
        _CACHE["consts"] = _host_consts()
    return _CACHE["nc"], _CACHE["consts"]


def _slice_moments(O, raw6, s):
    """Extract (T0, S0, Ty, Sy, Syy, Tx, Sx, Sxx) for slice s."""
    i, q = s // SPB, s % SPB
    p0 = 32 * q
    if USE_RAW6 and i == NBANK - 1:
        xp = np.arange(W, dtype=np.float64) - (W - 1) / 2.0
        r = raw6[p0 : p0 + 5].astype(np.float64)
        return (r[0].sum(), r[1].sum(), r[2].sum(), r[3].sum(), r[4].sum(),
                (r[0] * xp).sum(), (r[1] * xp).sum(), (r[1] * xp * xp).sum())
    return (O[p0 + 0, 3 * i], O[p0 + 1, 3 * i], O[p0 + 2, 3 * i],
            O[p0 + 3, 3 * i], O[p0 + 4, 3 * i], O[p0 + 0, 3 * i + 1],
            O[p0 + 1, 3 * i + 1], O[p0 + 1, 3 * i + 2])


def _combine(outs_per_core):
    loss_sum = 0.0
    n_valid = 0
    for O, raw6 in outs_per_core:
        O = np.asarray(O, dtype=np.float64)
        for s in range(S):
            T0, S0, Ty, Sy, Syy, Tx, Sx, Sxx = _slice_moments(O, raw6, s)
            if T0 > 0:
                cy = Ty / T0
                cx = Tx / T0
                loss_sum += (
                    (Syy - 2.0 * cy * Sy + cy * cy * S0)
                    + (Sxx - 2.0 * cx * Sx + cx * cx * S0)
                ) / HW
                n_valid += 1
    if n_valid > 0:
        return np.float32(loss_sum / n_valid)
    return np.float32(0.0)


def _in_maps(pred, target, consts):
    if USE_IOTA:
        base = {"wmom": consts}
    else:
        base = {"wmom": consts[0], "xc": consts[1], "xc2": consts[2]}
    maps = []
    for i in range(NCORES):
        maps.append(
            {
                "pred": pred[i * BPC : (i + 1) * BPC].reshape(S, H, W),
                "target": target[i * BPC : (i + 1) * BPC].reshape(S, H, W),
                **base,
            }
        )
    return maps


def kernel(pred, target):
    pred = np.ascontiguousarray(np.asarray(pred, dtype=np.float32))
    target = np.ascontiguousarray(np.asarray(target, dtype=np.float32))
    assert pred.shape == (B, C, H, W) and target.shape == (B, C, H, W)

    nc, consts = _get_built()
    res = run_bass_kernel_spmd(nc, _in_maps(pred, target, consts),
                               list(range(NCORES)))
    outs = [
        (res.results[i]["moments"], res.results[i].get("raw6"))
        for i in range(NCORES)
    ]
    return np.asarray(_combine(outs), dtype=np.float32)


if __name__ == "__main__":
    rng = np.random.default_rng(0)
    p = rng.standard_normal((B, C, H, W), dtype=np.float32)
    t = rng.random((B, C, H, W), dtype=np.float32)
    print(kernel(pred=p, target=t))
